# revision 12
# baseline (speedup 1.0000x reference)
"""CrossAttention2D TRN2 Bass kernel — data-parallel over batch on 8 NeuronCores.

Per core (one batch element), computed fully transposed ("feature-major"):
  qT[j,n]  = Wq'[c,j].T @ x[c,n]            (Wq' = Wq*scale; bias via DVE add)
  kT[j,t]  = Wk+[c,j].T @ ctxT+[c,t]        (bias folded as extra contraction row)
  v[t,j]   = ctxT+[c,t].T @ Wv+[c,j]
  ST[t,n]  = kT_h[d,t].T @ qT_h[d,n]        (per head, K=64)
  PT[t,n]  = exp(ST + maskbias[t])          (mask folds into ACT per-partition bias)
  OT       = [1(x64) | v_h][t,128].T @ PT[t,n]
             rows 0:64 = softmax denominator replicated, rows 64:128 = raw O
  oT[d,n]  = OT[64:128] * recip_fast(OT[0:64])   (one DVE op each, no broadcasts)
  outT[co,n] = Wo[j,co].T @ oT[j,n] + bo
Matmul dtype: fp16 (default; ~1e-3 L2) or float32r (~4e-4 L2, 2x DMA bytes).
"""

import sys

for _p in ("/opt/trn_rl_repo", "/opt/pypackages"):
    if _p not in sys.path:
        sys.path.append(_p)

import numpy as np

import concourse.bass as bass
import concourse.tile as tile
from concourse import bacc, mybir
from concourse.bass_utils import run_bass_kernel_spmd

F32 = mybir.dt.float32
F16 = mybir.dt.float16
F32R = mybir.dt.float32r

B = 8
C = 768            # DIM (q/out features)
CTX = 1024         # CTX_DIM
T = 256            # context tokens
N = 1024           # image tokens (32*32)
NH = 12            # heads
D = 64             # head dim
P = 128
CC = C // P        # 6
TC = T // P        # 2
MASK_NEG = -30.0

_NC_CACHE = {}


def _build_nc(use_fp16=True, with_bias=False):
    key = ("fp16" if use_fp16 else "f32r", with_bias)
    if key in _NC_CACHE:
        return _NC_CACHE[key]

    # k/v contraction length: +1 bias row zero-padded to a 128 multiple
    CKA = 1152 if with_bias else 1024
    KC = CKA // P
    MDT = F16 if use_fp16 else F32R      # matmul operand dtype
    DDT = F16 if use_fp16 else F32       # dram dtype for matmul inputs

    nc = bacc.Bacc("TRN2", target_bir_lowering=False, debug=False)

    x_d = nc.dram_tensor("x", [C, N], DDT, kind="ExternalInput")
    ctxT_d = nc.dram_tensor("ctxT", [CKA, T], DDT, kind="ExternalInput")
    mb_d = nc.dram_tensor("mb", [P, TC], F32, kind="ExternalInput")
    wq_d = nc.dram_tensor("wq", [C, C], DDT, kind="ExternalInput")
    wk_d = nc.dram_tensor("wk", [CKA, C], DDT, kind="ExternalInput")
    wv_d = nc.dram_tensor("wv", [CKA, C], DDT, kind="ExternalInput")
    wo_d = nc.dram_tensor("wo", [C, C], DDT, kind="ExternalInput")
    bq_d = bo_d = None
    if with_bias:
        bq_d = nc.dram_tensor("bq", [P, CC], F32, kind="ExternalInput")
        bo_d = nc.dram_tensor("bo", [P, CC], F32, kind="ExternalInput")
    out_d = nc.dram_tensor("out", [C, N], F32, kind="ExternalOutput")

    def mv(ap):  # matmul-view of a dram AP
        return ap if use_fp16 else ap.bitcast(F32R)

    # fp16 fits everything in SBUF at once; f32r needs the stage-1 inputs
    # freed before the attention-phase pools open.
    nbuf = 4 if use_fp16 else 2
    with tile.TileContext(nc) as tc:
        with (
            tc.tile_pool(name="consts", bufs=1) as cpool,
            tc.tile_pool(name="acts", bufs=1) as apool,
            tc.tile_pool(name="s1", bufs=1) as s1pool,
            tc.tile_pool(name="pt", bufs=nbuf) as ptpool,
            tc.tile_pool(name="rec", bufs=nbuf if use_fp16 else 1) as rpool,
            tc.tile_pool(name="outsb", bufs=3 if use_fp16 else 1) as opool,
            tc.tile_pool(name="psum", bufs=8, space="PSUM") as psum,
        ):
            # ---- DMA: q-path (wq/x kc-chunk pairs) split across queues,
            # then the k/v stream, then wo.
            wq_sb = s1pool.tile([P, CC, C], MDT)
            x_sb = s1pool.tile([P, CC, N], MDT)
            wq_r = mv(wq_d.rearrange("(k p) m -> p k m", p=P))
            x_r = mv(x_d.rearrange("(k p) n -> p k n", p=P))
            qeng = [nc.sync, nc.scalar, nc.sync, nc.scalar, nc.sync, nc.scalar]
            for kc in range(CC):
                qeng[kc].dma_start(wq_sb[:, kc, :], wq_r[:, kc, :])
                qeng[kc].dma_start(x_sb[:, kc, :], x_r[:, kc, :])
            ctxT_sb = s1pool.tile([P, KC, T], MDT)
            nc.scalar.dma_start(
                ctxT_sb[:], mv(ctxT_d.rearrange("(k p) t -> p k t", p=P)))
            wk_sb = s1pool.tile([P, KC, C], MDT)
            nc.scalar.dma_start(
                wk_sb[:], mv(wk_d.rearrange("(k p) m -> p k m", p=P)))
            wv_sb = s1pool.tile([P, KC, C], MDT)
            nc.sync.dma_start(
                wv_sb[:], mv(wv_d.rearrange("(k p) m -> p k m", p=P)))
            wo_sb = s1pool.tile([P, CC, C], MDT)
            nc.scalar.dma_start(
                wo_sb[:], mv(wo_d.rearrange("(k p) m -> p k m", p=P)))
            # ---- tiny consts on gpsimd SWDGE queue ----
            bq_sb = bo_sb = None
            if with_bias:
                bq_sb = cpool.tile([P, CC], F32)
                nc.gpsimd.dma_start(bq_sb[:], bq_d[:])
                bo_sb = cpool.tile([P, CC], F32)
                nc.gpsimd.dma_start(bo_sb[:], bo_d[:])
            mb_sb = cpool.tile([P, TC], F32)
            nc.gpsimd.dma_start(mb_sb[:], mb_d[:])
            ones_sb = cpool.tile([P, 1], F32)
            nc.vector.memset(ones_sb[:], 1.0)

            qT = apool.tile([P, CC, N], MDT)
            kT = apool.tile([P, CC, T], MDT)
            vpad = apool.tile([P, TC, NH, P], MDT)
            oT = apool.tile([P, CC, N], MDT)

            # vpad cols 0:64 <- 1.0 (denominator-replication trick)
            nc.vector.tensor_copy(
                vpad[:, :, :, 0:D],
                ones_sb[:, None, None, :].to_broadcast([P, TC, NH, D]),
            )

            # ---- stage 1a: qT = Wq'.T @ x (+bq); kc-outer for early start ----
            for g in range(2):           # jc groups of 3
                jcs = range(g * 3, g * 3 + 3)
                pss = {(jc, nh): psum.tile([P, 512], F32, tag="mm",
                                           name=f"q{jc}_{nh}")
                       for jc in jcs for nh in range(2)}
                for kc in range(CC):
                    for jc in jcs:
                        for nh in range(2):
                            nc.tensor.matmul(
                                pss[(jc, nh)][:],
                                wq_sb[:, kc, jc * P:(jc + 1) * P],
                                x_sb[:, kc, nh * 512:(nh + 1) * 512],
                                start=(kc == 0),
                                stop=(kc == CC - 1),
                            )
                for jc in jcs:
                    for nh in range(2):
                        dst = qT[:, jc, nh * 512:(nh + 1) * 512]
                        if with_bias:
                            nc.vector.tensor_scalar_add(
                                dst, pss[(jc, nh)][:], bq_sb[:, jc:jc + 1])
                        else:
                            nc.scalar.copy(dst, pss[(jc, nh)][:])

            # ---- stage 1b: kT = Wk+.T @ ctxT+ ----
            for jc in range(CC):
                ps = psum.tile([P, T], F32, tag="mm")
                for kc in range(KC):
                    nc.tensor.matmul(
                        ps[:],
                        wk_sb[:, kc, jc * P:(jc + 1) * P],
                        ctxT_sb[:, kc, :],
                        start=(kc == 0),
                        stop=(kc == KC - 1),
                    )
                nc.scalar.copy(kT[:, jc, :], ps[:])

            # ---- stage 1c: v = ctxT+.T @ Wv+ into vpad[:, tc, h, 64:128] ----
            for tcc in range(TC):
                for jh in range(2):
                    ps = psum.tile([P, 384], F32, tag="mm")
                    for kc in range(KC):
                        nc.tensor.matmul(
                            ps[:],
                            ctxT_sb[:, kc, tcc * P:(tcc + 1) * P],
                            wv_sb[:, kc, jh * 384:(jh + 1) * 384],
                            start=(kc == 0),
                            stop=(kc == KC - 1),
                        )
                    nc.scalar.copy(
                        vpad[:, tcc, jh * 6:(jh + 1) * 6, D:P],
                        ps[:].rearrange("p (h d) -> p h d", d=D),
                    )

            # ---- stage 2: attention per head ----
            for h in range(NH):
                r0 = (D * h) % P
                jc = h // 2
                pt = ptpool.tile([P, TC, N], MDT, tag="pt")
                for tcc in range(TC):
                    for nh in range(2):
                        st = psum.tile([P, 512], F32, tag="mm")
                        nc.tensor.matmul(
                            st[:],
                            kT[r0:r0 + D, jc, tcc * P:(tcc + 1) * P],
                            qT[r0:r0 + D, jc, nh * 512:(nh + 1) * 512],
                            start=True,
                            stop=True,
                        )
                        nc.scalar.activation(
                            pt[:, tcc, nh * 512:(nh + 1) * 512],
                            st[:],
                            mybir.ActivationFunctionType.Exp,
                            bias=mb_sb[:, tcc:tcc + 1],
                        )
                ots = [psum.tile([P, 512], F32, tag="mm", name=f"ot{i}")
                       for i in range(2)]
                for tcc in range(TC):
                    for nh in range(2):
                        nc.tensor.matmul(
                            ots[nh][:],
                            vpad[:, tcc, h, :],
                            pt[:, tcc, nh * 512:(nh + 1) * 512],
                            start=(tcc == 0),
                            stop=(tcc == TC - 1),
                        )
                rec = rpool.tile([D, N], F32, tag="rec")
                for nh in range(2):
                    nc.vector.reciprocal_approx_fast(
                        rec[:, nh * 512:(nh + 1) * 512], ots[nh][0:D, :])
                for nh in range(2):
                    nc.vector.tensor_mul(
                        oT[r0:r0 + D, jc, nh * 512:(nh + 1) * 512],
                        ots[nh][D:P, :],
                        rec[:, nh * 512:(nh + 1) * 512],
                    )

            # ---- stage 3: outT = Wo.T @ oT + bo ----
            for oc in range(CC):
                pss = [psum.tile([P, 512], F32, tag="mm", name=f"o{i}")
                       for i in range(2)]
                for jc in range(CC):
                    for nh in range(2):
                        nc.tensor.matmul(
                            pss[nh][:],
                            wo_sb[:, jc, oc * P:(oc + 1) * P],
                            oT[:, jc, nh * 512:(nh + 1) * 512],
                            start=(jc == 0),
                            stop=(jc == CC - 1),
                        )
                ob = opool.tile([P, N], F32, tag="ob")
                for nh in range(2):
                    if with_bias:
                        nc.vector.tensor_scalar_add(
                            ob[:, nh * 512:(nh + 1) * 512], pss[nh][:],
                            bo_sb[:, oc:oc + 1],
                        )
                    else:
                        nc.vector.tensor_copy(
                            ob[:, nh * 512:(nh + 1) * 512], pss[nh][:])
                    nc.sync.dma_start(
                        out_d.rearrange("(k p) n -> p k n", p=P)[
                            :, oc, nh * 512:(nh + 1) * 512],
                        ob[:, nh * 512:(nh + 1) * 512],
                    )

    nc.compile()
    _NC_CACHE[key] = nc
    return nc


def _prep_shared(Wq, bq, Wk, bk, Wv, bv, Wo, bo, np_dt, with_bias, CKA):
    scale = float(D) ** -0.5
    wq = np.ascontiguousarray(Wq * scale).astype(np_dt)
    wk = np.zeros((CKA, C), dtype=np_dt)
    wk[:CTX] = Wk.astype(np_dt)
    wv = np.zeros((CKA, C), dtype=np_dt)
    wv[:CTX] = Wv.astype(np_dt)
    wo = np.ascontiguousarray(Wo).astype(np_dt)
    bq_s = bo_s = None
    if with_bias:
        wk[CTX] = bk.astype(np_dt)
        wv[CTX] = bv.astype(np_dt)
        bq_s = np.ascontiguousarray(
            (bq * scale).reshape(CC, P).T, dtype=np.float32)
        bo_s = np.ascontiguousarray(bo.reshape(CC, P).T, dtype=np.float32)
    return wq, bq_s, wk, wv, wo, bo_s


def kernel(x, context, context_mask, Wq, bq, Wk, bk, Wv, bv, Wo, bo,
           _trace=False, _fp16=True):
    np_dt = np.float16 if _fp16 else np.float32
    x = np.asarray(x, dtype=np.float32)
    context = np.asarray(context, dtype=np.float32)
    context_mask = np.asarray(context_mask)
    bq, bk, bv, bo = (np.asarray(a) for a in (bq, bk, bv, bo))
    with_bias = any(np.any(a != 0) for a in (bq, bk, bv, bo))
    CKA = 1152 if with_bias else 1024
    wq, bq_s, wk, wv, wo, bo_s = _prep_shared(
        np.asarray(Wq), bq, np.asarray(Wk), bk,
        np.asarray(Wv), bv, np.asarray(Wo), bo, np_dt, with_bias, CKA,
    )

    in_maps = []
    for b in range(B):
        ctxT = np.zeros((CKA, T), dtype=np_dt)
        ctxT[:CTX] = context[b].T.astype(np_dt)
        if with_bias:
            ctxT[CTX] = 1.0
        mb = np.where(context_mask[b] != 0, 0.0, MASK_NEG).astype(np.float32)
        im = {
            "x": np.ascontiguousarray(x[b].reshape(C, N)).astype(np_dt),
            "ctxT": ctxT,
            "mb": np.ascontiguousarray(mb.reshape(TC, P).T),
            "wq": wq, "wk": wk, "wv": wv, "wo": wo,
        }
        if with_bias:
            im["bq"] = bq_s
            im["bo"] = bo_s
        in_maps.append(im)

    nc = _build_nc(use_fp16=_fp16, with_bias=with_bias)
    res = run_bass_kernel_spmd(nc, in_maps, list(range(B)), trace=_trace)
    out = np.stack([res.results[b]["out"].reshape(C, 32, 32) for b in range(B)])
    if _trace:
        kernel.last_exec_time_ns = res.exec_time_ns
        kernel.last_results = res
    return out


# revision 13
# speedup vs baseline: 1.0169x; 1.0169x over previous
"""CrossAttention2D TRN2 Bass kernel — data-parallel over batch on 8 NeuronCores.

Per core (one batch element), computed fully transposed ("feature-major"):
  qT[j,n]  = Wq'[c,j].T @ x[c,n]            (Wq' = Wq*scale; bias via DVE add)
  kT[j,t]  = Wk+[c,j].T @ ctxT+[c,t]        (bias folded as extra contraction row)
  v[t,j]   = ctxT+[c,t].T @ Wv+[c,j]
  ST[t,n]  = kT_h[d,t].T @ qT_h[d,n]        (per head, K=64)
  PT[t,n]  = exp(ST + maskbias[t])          (mask folds into ACT per-partition bias)
  OT       = [1(x64) | v_h][t,128].T @ PT[t,n]
             rows 0:64 = softmax denominator replicated, rows 64:128 = raw O
  oT[d,n]  = OT[64:128] * recip_fast(OT[0:64])   (one DVE op each, no broadcasts)
  outT[co,n] = Wo[j,co].T @ oT[j,n] + bo
Matmul dtype: fp16 (default; ~1e-3 L2) or float32r (~4e-4 L2, 2x DMA bytes).
"""

import sys

for _p in ("/opt/trn_rl_repo", "/opt/pypackages"):
    if _p not in sys.path:
        sys.path.append(_p)

import numpy as np

import concourse.bass as bass
import concourse.tile as tile
from concourse import bacc, mybir
from concourse.bass_utils import run_bass_kernel_spmd

F32 = mybir.dt.float32
F16 = mybir.dt.float16
F32R = mybir.dt.float32r

B = 8
C = 768            # DIM (q/out features)
CTX = 1024         # CTX_DIM
T = 256            # context tokens
N = 1024           # image tokens (32*32)
NH = 12            # heads
D = 64             # head dim
P = 128
CC = C // P        # 6
TC = T // P        # 2
MASK_NEG = -30.0

_NC_CACHE = {}


def _build_nc(use_fp16=True, with_bias=False):
    key = ("fp16" if use_fp16 else "f32r", with_bias)
    if key in _NC_CACHE:
        return _NC_CACHE[key]

    # k/v contraction length: +1 bias row zero-padded to a 128 multiple
    CKA = 1152 if with_bias else 1024
    KC = CKA // P
    MDT = F16 if use_fp16 else F32R      # matmul operand dtype
    DDT = F16 if use_fp16 else F32       # dram dtype for matmul inputs

    nc = bacc.Bacc("TRN2", target_bir_lowering=False, debug=False)

    x_d = nc.dram_tensor("x", [C, N], DDT, kind="ExternalInput")
    ctxT_d = nc.dram_tensor("ctxT", [CKA, T], DDT, kind="ExternalInput")
    mb_d = nc.dram_tensor("mb", [P, TC], F32, kind="ExternalInput")
    wq_d = nc.dram_tensor("wq", [C, C], DDT, kind="ExternalInput")
    wk_d = nc.dram_tensor("wk", [CKA, C], DDT, kind="ExternalInput")
    wv_d = nc.dram_tensor("wv", [CKA, C], DDT, kind="ExternalInput")
    wo_d = nc.dram_tensor("wo", [C, C], DDT, kind="ExternalInput")
    bq_d = bo_d = None
    if with_bias:
        bq_d = nc.dram_tensor("bq", [P, CC], F32, kind="ExternalInput")
        bo_d = nc.dram_tensor("bo", [P, CC], F32, kind="ExternalInput")
    out_d = nc.dram_tensor("out", [C, N], F32, kind="ExternalOutput")

    def mv(ap):  # matmul-view of a dram AP
        return ap if use_fp16 else ap.bitcast(F32R)

    # fp16 fits everything in SBUF at once; f32r needs the stage-1 inputs
    # freed before the attention-phase pools open.
    nbuf = 4 if use_fp16 else 2
    with tile.TileContext(nc) as tc:
        with (
            tc.tile_pool(name="consts", bufs=1) as cpool,
            tc.tile_pool(name="acts", bufs=1) as apool,
            tc.tile_pool(name="s1", bufs=1) as s1pool,
            tc.tile_pool(name="pt", bufs=nbuf) as ptpool,
            tc.tile_pool(name="rec", bufs=nbuf if use_fp16 else 1) as rpool,
            tc.tile_pool(name="outsb", bufs=3 if use_fp16 else 1) as opool,
            tc.tile_pool(name="psum", bufs=8, space="PSUM") as psum,
        ):
            # ---- DMA: q-path (wq/x kc-chunk pairs) split across queues,
            # then the k/v stream, then wo.
            wq_sb = s1pool.tile([P, CC, C], MDT)
            x_sb = s1pool.tile([P, CC, N], MDT)
            wq_r = mv(wq_d.rearrange("(k p) m -> p k m", p=P))
            x_r = mv(x_d.rearrange("(k p) n -> p k n", p=P))
            qeng = [nc.sync, nc.scalar, nc.sync, nc.scalar, nc.sync, nc.scalar]
            for kc in range(CC):
                qeng[kc].dma_start(wq_sb[:, kc, :], wq_r[:, kc, :])
                qeng[kc].dma_start(x_sb[:, kc, :], x_r[:, kc, :])
            ctxT_sb = s1pool.tile([P, KC, T], MDT)
            nc.scalar.dma_start(
                ctxT_sb[:], mv(ctxT_d.rearrange("(k p) t -> p k t", p=P)))
            wk_sb = s1pool.tile([P, KC, C], MDT)
            nc.scalar.dma_start(
                wk_sb[:], mv(wk_d.rearrange("(k p) m -> p k m", p=P)))
            wv_sb = s1pool.tile([P, KC, C], MDT)
            nc.sync.dma_start(
                wv_sb[:], mv(wv_d.rearrange("(k p) m -> p k m", p=P)))
            wo_sb = s1pool.tile([P, CC, C], MDT)
            nc.scalar.dma_start(
                wo_sb[:], mv(wo_d.rearrange("(k p) m -> p k m", p=P)))
            # ---- tiny consts on gpsimd SWDGE queue ----
            bq_sb = bo_sb = None
            if with_bias:
                bq_sb = cpool.tile([P, CC], F32)
                nc.gpsimd.dma_start(bq_sb[:], bq_d[:])
                bo_sb = cpool.tile([P, CC], F32)
                nc.gpsimd.dma_start(bo_sb[:], bo_d[:])
            mb_sb = cpool.tile([P, TC], F32)
            nc.gpsimd.dma_start(mb_sb[:], mb_d[:])
            ones_sb = cpool.tile([P, 1], F32)
            nc.vector.memset(ones_sb[:], 1.0)

            qT = apool.tile([P, CC, N], MDT)
            kT = apool.tile([P, CC, T], MDT)
            vpad = apool.tile([P, TC, NH, P], MDT)
            oT = apool.tile([P, CC, N], MDT)

            # vpad cols 0:64 <- 1.0 (denominator-replication trick)
            nc.vector.tensor_copy(
                vpad[:, :, :, 0:D],
                ones_sb[:, None, None, :].to_broadcast([P, TC, NH, D]),
            )

            # ---- stage 1a: qT = Wq'.T @ x (+bq); kc-outer for early start ----
            for g in range(2):           # jc groups of 3
                jcs = range(g * 3, g * 3 + 3)
                pss = {(jc, nh): psum.tile([P, 512], F32, tag="mm",
                                           name=f"q{jc}_{nh}")
                       for jc in jcs for nh in range(2)}
                for kc in range(CC):
                    for jc in jcs:
                        for nh in range(2):
                            nc.tensor.matmul(
                                pss[(jc, nh)][:],
                                wq_sb[:, kc, jc * P:(jc + 1) * P],
                                x_sb[:, kc, nh * 512:(nh + 1) * 512],
                                start=(kc == 0),
                                stop=(kc == CC - 1),
                            )
                for jc in jcs:
                    for nh in range(2):
                        dst = qT[:, jc, nh * 512:(nh + 1) * 512]
                        if with_bias:
                            nc.vector.tensor_scalar_add(
                                dst, pss[(jc, nh)][:], bq_sb[:, jc:jc + 1])
                        else:
                            nc.scalar.copy(dst, pss[(jc, nh)][:])

            # ---- stage 1b: kT = Wk+.T @ ctxT+ ----
            for jc in range(CC):
                ps = psum.tile([P, T], F32, tag="mm")
                for kc in range(KC):
                    nc.tensor.matmul(
                        ps[:],
                        wk_sb[:, kc, jc * P:(jc + 1) * P],
                        ctxT_sb[:, kc, :],
                        start=(kc == 0),
                        stop=(kc == KC - 1),
                    )
                nc.scalar.copy(kT[:, jc, :], ps[:])

            # ---- stage 1c: v = ctxT+.T @ Wv+ into vpad[:, tc, h, 64:128] ----
            for tcc in range(TC):
                for jh in range(2):
                    ps = psum.tile([P, 384], F32, tag="mm")
                    for kc in range(KC):
                        nc.tensor.matmul(
                            ps[:],
                            ctxT_sb[:, kc, tcc * P:(tcc + 1) * P],
                            wv_sb[:, kc, jh * 384:(jh + 1) * 384],
                            start=(kc == 0),
                            stop=(kc == KC - 1),
                        )
                    nc.scalar.copy(
                        vpad[:, tcc, jh * 6:(jh + 1) * 6, D:P],
                        ps[:].rearrange("p (h d) -> p h d", d=D),
                    )

            # ---- stage 2: attention per head ----
            for h in range(NH):
                r0 = (D * h) % P
                jc = h // 2
                pt = ptpool.tile([P, TC, N], MDT, tag="pt")
                for tcc in range(TC):
                    for nh in range(2):
                        st = psum.tile([P, 512], F32, tag="mm")
                        nc.tensor.matmul(
                            st[:],
                            kT[r0:r0 + D, jc, tcc * P:(tcc + 1) * P],
                            qT[r0:r0 + D, jc, nh * 512:(nh + 1) * 512],
                            start=True,
                            stop=True,
                        )
                        nc.scalar.activation(
                            pt[:, tcc, nh * 512:(nh + 1) * 512],
                            st[:],
                            mybir.ActivationFunctionType.Exp,
                            bias=mb_sb[:, tcc:tcc + 1],
                        )
                ots = [psum.tile([P, 512], F32, tag="mm", name=f"ot{i}")
                       for i in range(2)]
                for tcc in range(TC):
                    for nh in range(2):
                        nc.tensor.matmul(
                            ots[nh][:],
                            vpad[:, tcc, h, :],
                            pt[:, tcc, nh * 512:(nh + 1) * 512],
                            start=(tcc == 0),
                            stop=(tcc == TC - 1),
                        )
                rec = rpool.tile([D, N], F32, tag="rec")
                for nh in range(2):
                    nc.vector.reciprocal_approx_fast(
                        rec[:, nh * 512:(nh + 1) * 512], ots[nh][0:D, :])
                for nh in range(2):
                    nc.vector.tensor_mul(
                        oT[r0:r0 + D, jc, nh * 512:(nh + 1) * 512],
                        ots[nh][D:P, :],
                        rec[:, nh * 512:(nh + 1) * 512],
                    )

            # ---- stage 3: outT = Wo.T @ oT + bo ----
            for oc in range(CC):
                pss = [psum.tile([P, 512], F32, tag="mm", name=f"o{i}")
                       for i in range(2)]
                for jc in range(CC):
                    for nh in range(2):
                        nc.tensor.matmul(
                            pss[nh][:],
                            wo_sb[:, jc, oc * P:(oc + 1) * P],
                            oT[:, jc, nh * 512:(nh + 1) * 512],
                            start=(jc == 0),
                            stop=(jc == CC - 1),
                        )
                ob = opool.tile([P, N], F32, tag="ob")
                for nh in range(2):
                    if with_bias:
                        nc.vector.tensor_scalar_add(
                            ob[:, nh * 512:(nh + 1) * 512], pss[nh][:],
                            bo_sb[:, oc:oc + 1],
                        )
                    else:
                        nc.vector.tensor_copy(
                            ob[:, nh * 512:(nh + 1) * 512], pss[nh][:])
                    nc.sync.dma_start(
                        out_d.rearrange("(k p) n -> p k n", p=P)[
                            :, oc, nh * 512:(nh + 1) * 512],
                        ob[:, nh * 512:(nh + 1) * 512],
                    )

    nc.compile()
    _NC_CACHE[key] = nc
    return nc


def _prep_shared(Wq, bq, Wk, bk, Wv, bv, Wo, bo, np_dt, with_bias, CKA):
    scale = float(D) ** -0.5
    wq = np.ascontiguousarray(Wq * scale).astype(np_dt)
    wk = np.zeros((CKA, C), dtype=np_dt)
    wk[:CTX] = Wk.astype(np_dt)
    wv = np.zeros((CKA, C), dtype=np_dt)
    wv[:CTX] = Wv.astype(np_dt)
    wo = np.ascontiguousarray(Wo).astype(np_dt)
    bq_s = bo_s = None
    if with_bias:
        wk[CTX] = bk.astype(np_dt)
        wv[CTX] = bv.astype(np_dt)
        bq_s = np.ascontiguousarray(
            (bq * scale).reshape(CC, P).T, dtype=np.float32)
        bo_s = np.ascontiguousarray(bo.reshape(CC, P).T, dtype=np.float32)
    return wq, bq_s, wk, wv, wo, bo_s


def kernel(x, context, context_mask, Wq, bq, Wk, bk, Wv, bv, Wo, bo,
           _trace=False, _fp16=True):
    np_dt = np.float16 if _fp16 else np.float32
    x = np.asarray(x, dtype=np.float32)
    context = np.asarray(context, dtype=np.float32)
    context_mask = np.asarray(context_mask)
    bq, bk, bv, bo = (np.asarray(a) for a in (bq, bk, bv, bo))
    with_bias = any(np.any(a != 0) for a in (bq, bk, bv, bo))
    CKA = 1152 if with_bias else 1024
    wq, bq_s, wk, wv, wo, bo_s = _prep_shared(
        np.asarray(Wq), bq, np.asarray(Wk), bk,
        np.asarray(Wv), bv, np.asarray(Wo), bo, np_dt, with_bias, CKA,
    )

    in_maps = []
    for b in range(B):
        ctxT = np.zeros((CKA, T), dtype=np_dt)
        ctxT[:CTX] = context[b].T.astype(np_dt)
        if with_bias:
            ctxT[CTX] = 1.0
        mb = np.where(context_mask[b] != 0, 0.0, MASK_NEG).astype(np.float32)
        im = {
            "x": np.ascontiguousarray(x[b].reshape(C, N)).astype(np_dt),
            "ctxT": ctxT,
            "mb": np.ascontiguousarray(mb.reshape(TC, P).T),
            "wq": wq, "wk": wk, "wv": wv, "wo": wo,
        }
        if with_bias:
            im["bq"] = bq_s
            im["bo"] = bo_s
        in_maps.append(im)

    nc = _build_nc(use_fp16=_fp16, with_bias=with_bias)
    try:
        res = run_bass_kernel_spmd(nc, in_maps, list(range(B)), trace=_trace)
    except Exception:
        # transient NRT_EXEC_UNIT_UNRECOVERABLE etc. — one retry
        res = run_bass_kernel_spmd(nc, in_maps, list(range(B)), trace=_trace)
    out = np.stack([res.results[b]["out"].reshape(C, 32, 32) for b in range(B)])
    if _trace:
        kernel.last_exec_time_ns = res.exec_time_ns
        kernel.last_results = res
    return out
